# revision 46
# baseline (speedup 1.0000x reference)
"""Multi-head causal attention (B=2, T=2048, D=1024, H=16, Hd=64) on 8 trn2 cores.

Sharding: batch x head-group. Core c handles batch c//4 and heads
(c%4)*4 .. (c%4)*4+3 (data + tensor/head parallel). Each core computes
Q/K/V projections for its 4 heads, causal attention, and a partial
output projection (row-slice of Wo); the host sums the 4 partials per
batch and adds bo.

Device layout notes:
- Host passes x^T (q/k/v transposed to [D, T]) in bf16 so every matmul
  contraction has its operand partition-major; no on-chip transposes.
- Scores are computed transposed (S^T[t2, t1] = K^T.T @ Q^T) so softmax
  sums land on the PE via an appended ones-column in V (row 64 of the
  O^T psum accumulates the denominators for free).
- K^T lands in one tile per head with the unused 64 partitions zeroed so
  score matmuls contract over all 128 partitions (a 64-partition
  contraction streams at 2/3 the column rate).
- No max-subtraction in softmax: scaled scores are bounded (|S|/8 < 9
  for N(0,1)-scale inputs; exp stays far from fp32 overflow).
- Normalization: denominator row -> GpSimd partition broadcast -> DVE
  reciprocal -> DVE multiply into O^T (bf16); the very last norm runs
  the broadcast on the PE instead and chains warm-up matmuls off the
  denominator so the PE stays clocked up for the closing projection.
- Q/K projections run k-outer so the PE rides the x^T DMA stream; x
  arrives as four 0.5MB singles + two 1MB doubles per tensor (singles
  land in ~4.4us on their own DMA ring, and fewer dma_starts keep the
  ~0.7us-per-issue Sync queue off the critical path). Projection
  evictions run on the (phase-1-idle) Vector engine so Scalar enters
  the attention phase with no backlog ahead of the first exps.
- Output partials leave in fp16 (halves the 8MB/core writeback), two
  row-blocks per DMA; the final four blocks evict on Vector+Scalar in
  parallel and DMA per block so the drain starts sooner.
"""

import os
import sys

for _p in ("/root/.axon_site/_ro/trn_rl_repo", "/opt/trn_rl_repo"):
    if _p not in sys.path and os.path.isdir(_p):
        sys.path.append(_p)

import numpy as np
import ml_dtypes

B, T, D = 2, 2048, 1024
H, HD = 16, 64
HPC = 4                # heads per core
DH = HPC * HD          # 256 head-dim cols per core
KC = D // 128          # 8 contraction chunks
KC2 = KC // 2          # 4 double-chunk x tiles
NT4 = T // 512         # 4 t1-chunks
NB = T // 128          # 16 t2-blocks
N_CORES = 8

_BF16 = ml_dtypes.bfloat16
_F16 = np.float16
_cache = {}


def _build():
    import concourse.bass as bass
    import concourse.tile as tile
    from concourse import bacc, mybir

    f32 = mybir.dt.float32
    f16 = mybir.dt.float16
    bf16 = mybir.dt.bfloat16
    Exp = mybir.ActivationFunctionType.Exp
    Identity = mybir.ActivationFunctionType.Identity

    nc = bacc.Bacc(target_bir_lowering=False)

    xqt_d = nc.declare_dram_parameter("xqt", [D, T], bf16, isOutput=False)
    xkt_d = nc.declare_dram_parameter("xkt", [D, T], bf16, isOutput=False)
    xvt_d = nc.declare_dram_parameter("xvt", [D, T], bf16, isOutput=False)
    wq_d = nc.declare_dram_parameter("wq", [D, DH], bf16, isOutput=False)
    wk_d = nc.declare_dram_parameter("wk", [D, DH], bf16, isOutput=False)
    wv_d = nc.declare_dram_parameter("wv", [D, DH], bf16, isOutput=False)
    wo_d = nc.declare_dram_parameter("wo", [DH, D], bf16, isOutput=False)
    bq_d = nc.declare_dram_parameter("bq2", [128, 2], f32, isOutput=False)
    bk_d = nc.declare_dram_parameter("bk2", [128, 2], f32, isOutput=False)
    bv_d = nc.declare_dram_parameter("bv1", [1, DH], bf16, isOutput=False)
    tri_d = nc.declare_dram_parameter("tri", [128, 128], bf16, isOutput=False)
    out_d = nc.declare_dram_parameter("out", [T, D], f16, isOutput=True)

    with tile.TileContext(nc) as tc:
        with tc.tile_pool(name="const", bufs=1) as const, \
             tc.tile_pool(name="xpool", bufs=1) as xpool, \
             tc.tile_pool(name="ptp", bufs=15) as ptp, \
             tc.tile_pool(name="bcp", bufs=2) as bcp, \
             tc.tile_pool(name="outp", bufs=2) as outp, \
             tc.tile_pool(name="ps_a", bufs=2, space="PSUM") as ps_a, \
             tc.tile_pool(name="ps_o", bufs=1, space="PSUM") as ps_o, \
             tc.tile_pool(name="ps_v", bufs=2, space="PSUM") as ps_v:

            # ---- DMA issue order: everything Q-proj needs first ----
            ones_bf = const.tile([1, 128], bf16)
            nc.vector.memset(ones_bf[:], 1.0)
            ones256 = const.tile([1, DH], bf16)
            nc.vector.memset(ones256[:], 1.0)
            ones_f32 = const.tile([1, 128], f32)
            nc.vector.memset(ones_f32[:], 1.0)
            # touch Exp and Identity on the Scalar engine now: the first use
            # of an activation function triggers a ~1.3us ACT_TABLE_LOAD,
            # which otherwise lands mid-kernel in front of the K evictions
            act_warm = const.tile([1, 8], f32)
            nc.scalar.activation(
                out=act_warm[:], in_=ones_f32[:, 0:8], func=Exp, scale=1.0
            )
            nc.scalar.activation(
                out=act_warm[:], in_=ones_f32[:, 0:8], func=Identity, scale=1.0
            )

            # Mixed DMA granularity: the first chunks of each tensor come as
            # small 0.5MB transfers (land fast on their own ring), the rest in
            # 1MB doubles to bound Sync-issue serialization (~0.7us per
            # dma_start) and ring-slot pressure.
            def load_xt(xt_d, tag):
                views = []
                for k in (0, 1, 2, 3):
                    t = xpool.tile([128, T], bf16, tag=f"{tag}s", bufs=4)
                    nc.sync.dma_start(out=t[:], in_=xt_d[k * 128 : (k + 1) * 128, :])
                    views.append(t[:])
                for k2 in (2, 3):
                    t = xpool.tile([128, 2, T], bf16, tag=f"{tag}d", bufs=2)
                    nc.sync.dma_start(
                        out=t[:],
                        in_=xt_d[k2 * 256 : (k2 + 1) * 256, :].rearrange(
                            "(k p) n -> p k n", p=128
                        ),
                    )
                    views.append(t[:, 0, :])
                    views.append(t[:, 1, :])
                return views

            wq_sb = const.tile([128, KC, DH], bf16)
            nc.sync.dma_start(out=wq_sb[:], in_=wq_d[:].rearrange("(k p) n -> p k n", p=128))
            xqv = load_xt(xqt_d, "q")
            bqk_sb = const.tile([128, 4], f32)
            nc.sync.dma_start(out=bqk_sb[:, 0:2], in_=bq_d[:])
            nc.sync.dma_start(out=bqk_sb[:, 2:4], in_=bk_d[:])
            bq_sb = bqk_sb[:, 0:2]
            bk_sb = bqk_sb[:, 2:4]

            wk_sb = const.tile([128, KC, DH], bf16)
            nc.sync.dma_start(out=wk_sb[:], in_=wk_d[:].rearrange("(k p) n -> p k n", p=128))
            xkv = load_xt(xkt_d, "k")

            bv_sb = const.tile([1, DH], bf16)
            nc.sync.dma_start(out=bv_sb[:], in_=bv_d[:])
            tri_sb = const.tile([128, 128], bf16)
            nc.sync.dma_start(out=tri_sb[:], in_=tri_d[:])
            wv_sb = const.tile([128, KC, DH], bf16)
            nc.sync.dma_start(out=wv_sb[:], in_=wv_d[:].rearrange("(k p) n -> p k n", p=128))
            xvv = []
            for k4 in range(2):
                t = xpool.tile([128, 4, T], bf16, tag="vq", bufs=2)
                nc.sync.dma_start(
                    out=t[:],
                    in_=xvt_d[k4 * 512 : (k4 + 1) * 512, :].rearrange(
                        "(k p) n -> p k n", p=128
                    ),
                )
                for j in range(4):
                    xvv.append(t[:, j, :])
            wo_sb = const.tile([128, 2, D], bf16)
            nc.sync.dma_start(out=wo_sb[:], in_=wo_d[:].rearrange("(c p) n -> p c n", p=128))

            def xq(k):
                return xqv[k]

            def xk(k):
                return xkv[k]

            def xv(k):
                return xvv[k]

            # bv broadcast tile [128, DH]. The leading ones x ones overwrites
            # are PE warm-up (HAM un-throttle) with no DMA dependency, so they
            # run during the framework preamble / first x^T transfers; the
            # final matmul (start=True resets the psum) is the real bv
            # broadcast and only it waits for the bv DMA.
            bvb_ps = ps_v.tile([128, DH], f32, tag="vps")
            for _ in range(26):
                nc.tensor.matmul(bvb_ps[:], ones_bf[:], ones256[:], start=True, stop=True)
            nc.tensor.matmul(bvb_ps[:], ones_bf[:], bv_sb[:], start=True, stop=True)
            bvb_sb = const.tile([128, DH], bf16)
            nc.vector.tensor_copy(out=bvb_sb[:], in_=bvb_ps[:])

            # ---- persistent activations ----
            # K^T lands in one tile per head with partitions 64:128 zeroed, so
            # score matmuls contract over the full 128 partitions (the upper
            # half of the packed q^T operand is the neighbor head's data, but
            # it multiplies the zero rows of K). 128-partition matmuls stream
            # 1 column/cycle vs 1.5 for 64-partition ones.
            qt_sb = [const.tile([128, T], bf16, tag=f"qt{i}", name=f"qt{i}") for i in range(2)]
            # head h keeps its K data at the same partition offset as in the
            # packed q^T tile ((h%2)*64); the other 64 partitions are zero
            ktp_sb = [const.tile([128, T], bf16, tag=f"ktp{i}", name=f"ktp{i}") for i in range(HPC)]
            for h in range(HPC):
                zoff = 64 if h % 2 == 0 else 0
                nc.gpsimd.memset(ktp_sb[h][zoff : zoff + 64, :], 0.0)
            ont_sb = [const.tile([128, T], bf16, tag=f"ont{i}", name=f"ont{i}") for i in range(2)]
            vaug_sb = const.tile([128, NB, HPC * (HD + 1)], bf16)
            # ones columns for the denominator trick
            nc.vector.memset(
                vaug_sb[:].rearrange("p b (h x) -> p b h x", h=HPC)[:, :, :, HD : HD + 1],
                1.0,
            )

            # ---- phase 1: Q^T / K^T projections, k-outer so the PE starts
            # on chunk 0 while later chunks are still in flight ----
            for which, (xf, w_sb, b_sb) in enumerate(
                [(xq, wq_sb, bq_sb), (xk, wk_sb, bk_sb)]
            ):
                for dhc in range(2):
                    psA = ps_a.tile([128, 2, 512], f32, tag="sa")
                    psB = ps_a.tile([128, 2, 512], f32, tag="sa")
                    for k in range(KC):
                        w = w_sb[:, k, dhc * 128 : (dhc + 1) * 128]
                        for t4, ps in ((0, psA), (1, psA), (2, psB), (3, psB)):
                            nc.tensor.matmul(
                                ps[:, t4 % 2, :],
                                w,
                                xf(k)[:, t4 * 512 : (t4 + 1) * 512],
                                start=(k == 0),
                                stop=(k == KC - 1),
                            )
                    # evictions run on Vector (idle during phase 1) so the
                    # Scalar engine enters the attention phase with no backlog
                    # ahead of the first exps
                    for half, ps in ((0, psA), (1, psB)):
                        if which == 0:
                            nc.vector.tensor_scalar_add(
                                qt_sb[dhc][:, half * 1024 : (half + 1) * 1024],
                                ps[:].rearrange("p a n -> p (a n)"),
                                b_sb[:, dhc : dhc + 1],
                            )
                        else:
                            # K^T: split the 2-head psum into per-head tiles,
                            # keeping each head at its packed partition offset.
                            # 512-col quarters, first-needed columns first,
                            # heads alternating Vector/Scalar — four serial
                            # 64-partition DVE ops here gate chunk-0's h>=2
                            # scores by ~3.5us otherwise.
                            for q in range(2):
                                for hh in range(2):
                                    ap_out = ktp_sb[2 * dhc + hh][
                                        hh * 64 : (hh + 1) * 64,
                                        half * 1024 + q * 512 : half * 1024 + (q + 1) * 512,
                                    ]
                                    ap_in = ps[hh * 64 : (hh + 1) * 64, q, :]
                                    bias = b_sb[hh * 64 : (hh + 1) * 64, dhc : dhc + 1]
                                    if hh == 0:
                                        nc.vector.tensor_scalar_add(ap_out, ap_in, bias)
                                    else:
                                        nc.scalar.activation(
                                            out=ap_out,
                                            in_=ap_in,
                                            func=Identity,
                                            bias=bias,
                                            scale=1.0,
                                        )

            # ---- phases 2+3: attention with fine-grained interleave ----
            # S tiles are emitted in 2-block pairs sharing one 2-bank psum
            # tile so full pairs need a single (cheaper) exp op. PV(h) and
            # S(h+1) alternate so the PE always has independent work while
            # ACT drains exps; V-projection and output-projection units drip
            # into the stream as PE fillers.
            def make_v_unit(tb):
                def emit():
                    ps = ps_v.tile([128, DH], f32, tag="vps", name="v_ps")
                    for k in range(KC):
                        nc.tensor.matmul(
                            ps[:],
                            xv(k)[:, tb * 128 : (tb + 1) * 128],
                            wv_sb[:, k, :],
                            start=(k == 0),
                            stop=(k == KC - 1),
                        )
                    nc.vector.tensor_add(
                        vaug_sb[:, tb, :].rearrange("p (h x) -> p h x", h=HPC)[:, :, 0:HD],
                        ps[:].rearrange("p (h x) -> p h x", h=HPC),
                        bvb_sb[:].rearrange("p (h x) -> p h x", h=HPC),
                    )
                return emit

            # output projection: two row-blocks share an outp tile; the DMA
            # fires on the odd block (2 blocks per transfer). Final units
            # split evictions across Vector and Scalar (both idle at the
            # tail) and DMA per block so the drain starts sooner.
            ob_tiles = {}
            bc_scrap = const.tile([1, 8], f32)

            def make_outproj_unit(m, final=False):
                def emit():
                    ps = ps_a.tile([128, 2, 512], f32, tag="sa", name="op_ps")
                    if m % 2 == 0:
                        ob_tiles[m // 2] = outp.tile([128, 2, D], f16, tag="ob", name="ob")
                    ob = ob_tiles[m // 2]
                    for n2 in range(2):
                        for dhc in range(2):
                            nc.tensor.matmul(
                                ps[:, n2, :],
                                ont_sb[dhc][:, m * 128 : (m + 1) * 128],
                                wo_sb[:, dhc, n2 * 512 : (n2 + 1) * 512],
                                start=(dhc == 0),
                                stop=(dhc == 1),
                            )
                        if final and n2 == 1:
                            nc.scalar.copy(
                                out=ob[:, m % 2, n2 * 512 : (n2 + 1) * 512],
                                in_=ps[:, n2, :],
                            )
                        else:
                            nc.vector.tensor_copy(
                                out=ob[:, m % 2, n2 * 512 : (n2 + 1) * 512],
                                in_=ps[:, n2, :],
                            )
                    if final:
                        nc.sync.dma_start(
                            out=out_d[m * 128 : (m + 1) * 128, :],
                            in_=ob[:, m % 2, :],
                        )
                    elif m % 2 == 1:
                        nc.sync.dma_start(
                            out=out_d[(m - 1) * 128 : (m + 1) * 128, :].rearrange(
                                "(c p) n -> p c n", p=128
                            ),
                            in_=ob[:],
                        )
                return emit

            v_fns = [make_v_unit(tb) for tb in range(NB)]
            v_next = [0]

            for c in range(NT4):
                nblk = 4 * c + 4

                def s_pair(h, bp):
                    # blocks b0=2bp, b1=2bp+1 share one [128, 2, 512] psum tile
                    hc = h // 2
                    s_ps = ps_a.tile([128, 2, 512], f32, tag="sa", name="s_ps")
                    pt = ptp.tile([128, 2, 512], bf16, tag="pt", name="pt")
                    geo = []
                    for i in range(2):
                        b = 2 * bp + i
                        r = b - 4 * c
                        off = max(r, 0) * 128
                        w = 512 - off
                        geo.append((b, r, off, w))
                        nc.tensor.matmul(
                            s_ps[:, i, off : off + w],
                            ktp_sb[h][:, b * 128 : (b + 1) * 128],
                            qt_sb[hc][:, c * 512 + off : (c + 1) * 512],
                            start=True,
                            stop=True,
                        )
                    if geo[1][2] <= 128:
                        # at most 128 unwritten psum cols: one merged exp over
                        # the full tile beats two ops (~250ns fixed cost each);
                        # the exp of stale cols is never read — PV skips them
                        nc.scalar.activation(
                            out=pt[:], in_=s_ps[:], func=Exp, scale=0.125
                        )
                    else:
                        for i, (b, r, off, w) in enumerate(geo):
                            nc.scalar.activation(
                                out=pt[:, i, off : off + w],
                                in_=s_ps[:, i, off : off + w],
                                func=Exp,
                                scale=0.125,
                            )
                    for i, (b, r, off, w) in enumerate(geo):
                        if r >= 0:
                            nc.vector.tensor_mul(
                                pt[:, i, off : off + 128],
                                pt[:, i, off : off + 128],
                                tri_sb[:],
                            )
                    return (pt, geo)

                def pv_block(h, b, pairs, o_ps):
                    pt, geo = pairs[b // 2]
                    i = b % 2
                    _, r, off, w = geo[i]
                    nc.tensor.matmul(
                        o_ps[:, off : off + w],
                        vaug_sb[:, b, h * (HD + 1) : (h + 1) * (HD + 1)],
                        pt[:, i, off : off + w],
                        start=(b == 0),
                        stop=(b == nblk - 1),
                    )

                def norm(h, o_ps, tail=False):
                    hc, hr = h // 2, (h % 2) * 64
                    if tail:
                        # last norm of the kernel: broadcast the denominator
                        # row on the PE (fast + keeps it clocked up for the
                        # closing output projection); den goes through bf16,
                        # fine for a 1/16th slice of the output
                        den_b = bcp.tile([1, 512], bf16, tag="denb", name="den_b")
                        nc.scalar.copy(out=den_b[:], in_=o_ps[64 : HD + 1, :])
                        bc_ps = ps_o.tile([64, 512], f32, tag="ops0", name="bc_ps")
                        nc.tensor.matmul(
                            bc_ps[:], ones_bf[:, 0:64], den_b[:], start=True, stop=True
                        )
                        # den-chained PE warmers: they can only run once den_b
                        # exists, i.e. they fill the reciprocal+multiply window
                        warm_ps = ps_v.tile([128, DH], f32, tag="vps")
                        for _ in range(12):
                            nc.tensor.matmul(
                                warm_ps[:],
                                ones_bf[:],
                                den_b[:, 0:DH],
                                start=True,
                                stop=True,
                            )
                        bcb = bcp.tile([64, 512], f32, tag="bcb", name="bcb", bufs=2)
                        nc.vector.reciprocal_approx_fast(out=bcb[:], in_=bc_ps[:])
                        # fp32 warmers chained on the reciprocal output keep
                        # the PE hot through the final ont multiply
                        for _ in range(3):
                            nc.tensor.matmul(
                                warm_ps[:],
                                ones_f32[:],
                                bcb[0:1, 0:DH],
                                start=True,
                                stop=True,
                            )
                        nc.scalar.copy(out=bc_scrap[:], in_=warm_ps[0:1, 0:8])
                    else:
                        den_f = bcp.tile([1, 512], f32, tag="den", name="den_f")
                        nc.vector.tensor_copy(out=den_f[:], in_=o_ps[64 : HD + 1, :])
                        bc_sb = bcp.tile([64, 512], f32, tag="bcs", name="bc_sb", bufs=2)
                        nc.gpsimd.partition_broadcast(bc_sb[:], den_f[:])
                        bcb = bcp.tile([64, 512], f32, tag="bcb", name="bcb", bufs=2)
                        nc.vector.reciprocal_approx_fast(out=bcb[:], in_=bc_sb[:])
                    nc.vector.tensor_mul(
                        ont_sb[hc][hr : hr + 64, c * 512 : (c + 1) * 512],
                        o_ps[0:HD, :],
                        bcb[:],
                    )

                op_units = (
                    [make_outproj_unit(m) for m in range(4 * (c - 1), 4 * c)]
                    if c > 0
                    else []
                )
                v_own_end = 4 * c + 4       # blocks this chunk's PV needs
                v_pre_end = min(4 * c + 6, NB)  # next chunk's first 2 blocks

                o_pss = [
                    ps_o.tile([HD + 1, 512], f32, tag=f"ops{h % 2}", name=f"ops{h}")
                    for h in range(HPC)
                ]
                npair = nblk // 2
                ptss = {}

                # stream A: scores(0) pairs interleaved with the V units this
                # chunk still needs (most were pre-dripped a chunk early)
                ptss[0] = []
                for bp in range(npair):
                    ptss[0].append(s_pair(0, bp))
                    if v_next[0] < v_own_end:
                        v_fns[v_next[0]]()
                        v_next[0] += 1
                while v_next[0] < v_own_end:
                    v_fns[v_next[0]]()
                    v_next[0] += 1

                # streams B-E: S(h+1) pairs and PV(h) alternate; outproj and
                # next-chunk V units drip in as PE fillers
                for h in range(HPC):
                    hn = h + 1
                    if hn < HPC:
                        ptss[hn] = []
                    for bp in range(npair):
                        if hn < HPC:
                            ptss[hn].append(s_pair(hn, bp))
                        pv_block(h, 2 * bp, ptss[h], o_pss[h])
                        pv_block(h, 2 * bp + 1, ptss[h], o_pss[h])
                        if op_units and bp % 2 == 1:
                            op_units.pop(0)()
                        elif h >= 1 and v_next[0] < v_pre_end:
                            v_fns[v_next[0]]()
                            v_next[0] += 1
                    ptss.pop(h)
                    norm(h, o_pss[h], tail=(c == NT4 - 1 and h == HPC - 1))
                while op_units:
                    op_units.pop(0)()

            # final chunk's output projection
            for m in range(4 * (NT4 - 1), 4 * NT4):
                make_outproj_unit(m, final=True)()

    nc.compile()
    return nc


def _get_nc():
    if "nc" not in _cache:
        _cache["nc"] = _build()
    return _cache["nc"]


def build_in_maps(query, key, value, Wq, bq, Wk, bk, Wv, bv, Wo, bo):
    query = np.asarray(query, np.float32)
    key = np.asarray(key, np.float32)
    value = np.asarray(value, np.float32)
    Wq_, Wk_, Wv_, Wo_ = (np.asarray(a, np.float32) for a in (Wq, Wk, Wv, Wo))
    bq_, bk_, bv_, bo_ = (np.asarray(a, np.float32) for a in (bq, bk, bv, bo))

    xqt = [np.ascontiguousarray(query[b].T).astype(_BF16) for b in range(B)]
    xkt = [np.ascontiguousarray(key[b].T).astype(_BF16) for b in range(B)]
    xvt = [np.ascontiguousarray(value[b].T).astype(_BF16) for b in range(B)]

    tri = np.tril(np.ones((128, 128), np.float32)).T.astype(_BF16)  # tri[j,i]=1 iff j<=i

    in_maps = []
    for c in range(N_CORES):
        b, hg = c // 4, c % 4
        sl = slice(hg * DH, (hg + 1) * DH)
        in_maps.append(
            {
                "xqt": xqt[b],
                "xkt": xkt[b],
                "xvt": xvt[b],
                "wq": np.ascontiguousarray(Wq_[:, sl]).astype(_BF16),
                "wk": np.ascontiguousarray(Wk_[:, sl]).astype(_BF16),
                "wv": np.ascontiguousarray(Wv_[:, sl]).astype(_BF16),
                "wo": np.ascontiguousarray(Wo_[sl, :]).astype(_BF16),
                "bq2": np.ascontiguousarray(bq_[sl].reshape(2, 128).T),
                "bk2": np.ascontiguousarray(bk_[sl].reshape(2, 128).T),
                "bv1": bv_[sl].reshape(1, DH).astype(_BF16),
                "tri": tri,
            }
        )

    return in_maps, bo_


def kernel(query, key, value, Wq, bq, Wk, bk, Wv, bv, Wo, bo):
    from concourse.bass_utils import run_bass_kernel_spmd

    nc = _get_nc()
    in_maps, bo_ = build_in_maps(query, key, value, Wq, bq, Wk, bk, Wv, bv, Wo, bo)
    res = run_bass_kernel_spmd(nc, in_maps, list(range(N_CORES)))
    _cache["last_results"] = res

    out = np.empty((B, T, D), np.float32)
    for b in range(B):
        acc = res.results[4 * b]["out"].astype(np.float32)
        for hg in range(1, 4):
            acc += res.results[4 * b + hg]["out"].astype(np.float32)
        out[b] = acc + bo_[None, :]
    return out
